# revision 9
# baseline (speedup 1.0000x reference)
"""Trainium2 Bass kernel for nn_Attention_88493506167116.

Channel-attention module (per batch item b):
    F = (Wf @ X).reshape raw (N, C);  G likewise;  Hm likewise (X = x[b] as (C, N))
    S = G^T @ F  (C x C), beta = softmax(S, axis=-1)
    O = beta @ Hm^T  (C, N) -> reshape (C, W, H);  out = Wo @ O + bo

Key structure (C=256, N=4096 = 16*C): the raw reshape (C, N) -> (N, C) is a
block regrouping: F_r[16c+q, r] = Yf[c, q*C + r].  Hence with X_q = X[:, qC:(q+1)C]:
    S     = sum_q Yg_q^T Yf_q = sum_q X_q^T A X_q,   A = Wg^T Wf   (host-folded)
    out   = Wo @ O: with P = Wo @ beta, Out[o, 16c+q] = (P @ Yh_q^T)[o, c]
and Yh_q^T = YhT[qC:(q+1)C, :] where YhT = X^T Wh^T is computed directly in
transposed layout (no on-device transposes anywhere).

Sharding: pure data-parallel, batch B=8 across the 8 NeuronCores (one image
per core), zero collectives.  Host folds A = Wf.T @ Wg (lhsT layout) and
transposes weights; all heavy compute is on-device fp32 matmuls.

Hardware constraints honored here: fp32 matmuls self-load weights (S3_LW) and
DMA instructions each carry at most ONE sync wait, so (a) all weights are
packed into one DRAM param (<=8 DMAs total -> no DMA sem-lane reuse), (b) tiny
warmup matmuls touch every DMA'd matmul input first, and (c) all PSUM
evacuations / matmul-input producers run on a single engine per path so later
waits coalesce onto one semaphore.
"""

import numpy as np

B, C, W_DIM, H_DIM = 8, 256, 64, 64
N = W_DIM * H_DIM          # 4096
Q = N // C                 # 16
P = 128                    # partitions
NCORES = 8

_GRAPH_CACHE = {}


def _build_graph(use_bias: bool):
    from contextlib import ExitStack

    import concourse.bass as bass
    import concourse.tile as tile
    from concourse import mybir

    f32 = mybir.dt.float32
    AF = mybir.ActivationFunctionType

    nc = bass.Bass()

    NW = 8 if use_bias else 6
    x_ext = nc.declare_dram_parameter("x", [C, N], f32, isOutput=False)
    wpk_ext = nc.declare_dram_parameter("wpk", [NW * P, C], f32, isOutput=False)
    if use_bias:
        bpk_ext = nc.declare_dram_parameter("bpk", [P, 6], f32, isOutput=False)
        bhw_ext = nc.declare_dram_parameter("bhw", [1, 2 * C], f32, isOutput=False)
    out_ext = nc.declare_dram_parameter("out", [C, N], f32, isOutput=True)

    with ExitStack() as ctx:
        tc = ctx.enter_context(tile.TileContext(nc))
        cpool = ctx.enter_context(tc.tile_pool(name="cpool", bufs=1))
        psS = ctx.enter_context(tc.tile_pool(name="psS", bufs=1, space="PSUM"))
        psW = ctx.enter_context(tc.tile_pool(name="psW", bufs=5, space="PSUM"))
        psX = ctx.enter_context(tc.tile_pool(name="psX", bufs=1, space="PSUM"))

        # single evacuation engine per path (see module docstring)
        if use_bias:
            def evac(dst, src):
                nc.scalar.copy(dst, src)
        else:
            def evac(dst, src):
                nc.vector.tensor_copy(dst, src)

        # ---- resident SBUF tensors (3-5 input DMAs total) --------------
        x_sb = [cpool.tile([P, N], f32, name=f"x{kc}") for kc in range(2)]
        wpk_sb = cpool.tile([P, NW, C], f32, name="wpk")
        for kc in range(2):
            nc.sync.dma_start(out=x_sb[kc][:], in_=x_ext[kc * P:(kc + 1) * P, :])
        nc.sync.dma_start(out=wpk_sb[:], in_=wpk_ext.rearrange("(g p) c -> p g c", p=P))

        if use_bias:
            wft_sb = [wpk_sb[:, 0 + kc, :] for kc in range(2)]
            wgt_sb = [wpk_sb[:, 2 + kc, :] for kc in range(2)]
            wht_sb = [wpk_sb[:, 4 + kc, :] for kc in range(2)]
            wot_sb = [wpk_sb[:, 6 + kc, :] for kc in range(2)]
            bpk_sb = cpool.tile([P, 6], f32, name="bpk")
            bhw_sb = cpool.tile([1, 2 * C], f32, name="bhw")
            nc.sync.dma_start(out=bpk_sb[:], in_=bpk_ext[:])
            nc.sync.dma_start(out=bhw_sb[:], in_=bhw_ext[:])
            bf_sb = [bpk_sb[:, 0 + kc:1 + kc] for kc in range(2)]
            bg_sb = [bpk_sb[:, 2 + kc:3 + kc] for kc in range(2)]
            bo_sb = [bpk_sb[:, 4 + kc:5 + kc] for kc in range(2)]
            bh_row = bhw_sb[0:1, 0:C]
            wosum_row = bhw_sb[0:1, C:2 * C]
        else:
            at_sb = [wpk_sb[:, 0 + kc, :] for kc in range(2)]
            wht_sb = [wpk_sb[:, 2 + kc, :] for kc in range(2)]
            wot_sb = [wpk_sb[:, 4 + kc, :] for kc in range(2)]

        # ---- PE sync warmup: one tiny matmul per DMA sem lane ----------
        # (fp32 matmuls take <=1 sync wait; these cover the DMA lanes so
        # real matmuls never need two DMA waits at once)
        scratch_ps = psX.tile([P, 1], f32, name="scratch")
        warm = [x_sb[0][:, 0:P], x_sb[1][:, 0:P], wpk_sb[:, 0, 0:P]]
        for t in warm:
            nc.tensor.matmul(scratch_ps[:], t, t[:, 0:1], start=True, stop=True)
        if use_bias:
            nc.tensor.matmul(scratch_ps[0:1, :], bhw_sb[0:1, 0:1],
                             bhw_sb[0:1, 0:1], start=True, stop=True)
            # pre-cover the bpk DMA lane on the ACT engine (bias reads)
            act_scr = cpool.tile([P, 1], f32, name="act_scr")
            nc.scalar.copy(act_scr[:], bpk_sb[:, 0:1])

        # S accumulator PSUM tiles, pinned across the whole contraction
        psS_t = [psS.tile([P, C], f32, name=f"S{ac}") for ac in range(2)]

        # ================================================================
        # Pre-softmax path: fill psS_t[ac] with S = G_r^T F_r
        # ================================================================
        if not use_bias:
            # T_q = A @ X_q, then S += X_q^T T_q  (A_T = Wf^T Wg passed in)
            t_sb = [[cpool.tile([P, C], f32, name=f"t{q}_{uc}") for uc in range(2)]
                    for q in range(Q)]
            for q in range(Q):
                for uc in range(2):
                    ps = psW.tile([P, C], f32, name="w")
                    for kc in range(2):
                        nc.tensor.matmul(
                            ps[:],
                            at_sb[kc][:, uc * P:(uc + 1) * P],
                            x_sb[kc][:, q * C:(q + 1) * C],
                            start=(kc == 0), stop=(kc == 1),
                        )
                    evac(t_sb[q][uc][:], ps[:])
            for ac in range(2):
                for q in range(Q):
                    for uc in range(2):
                        nc.tensor.matmul(
                            psS_t[ac][:],
                            x_sb[uc][:, q * C + ac * P: q * C + ac * P + P],
                            t_sb[q][uc][:],
                            start=(q == 0 and uc == 0),
                            stop=(q == Q - 1 and uc == 1),
                        )
        else:
            # materialize Yf = Wf X + bf and Yg = Wg X + bg, then
            # S = sum_q Yg_q^T Yf_q
            yf_sb = [cpool.tile([P, N], f32, name=f"yf{mc}") for mc in range(2)]
            yg_sb = [cpool.tile([P, N], f32, name=f"yg{mc}") for mc in range(2)]
            for mc in range(2):
                for nb in range(8):
                    nsl = slice(nb * 512, (nb + 1) * 512)
                    ps = psW.tile([P, 512], f32, name="w")
                    for kc in range(2):
                        nc.tensor.matmul(
                            ps[:], wft_sb[kc][:, mc * P:(mc + 1) * P],
                            x_sb[kc][:, nsl], start=(kc == 0), stop=(kc == 1))
                    nc.scalar.activation(yf_sb[mc][:, nsl], ps[:], AF.Identity,
                                         bias=bf_sb[mc], scale=1.0)
                    ps = psW.tile([P, 512], f32, name="w")
                    for kc in range(2):
                        nc.tensor.matmul(
                            ps[:], wgt_sb[kc][:, mc * P:(mc + 1) * P],
                            x_sb[kc][:, nsl], start=(kc == 0), stop=(kc == 1))
                    nc.scalar.activation(yg_sb[mc][:, nsl], ps[:], AF.Identity,
                                         bias=bg_sb[mc], scale=1.0)
            for ac in range(2):
                for q in range(Q):
                    for kc in range(2):
                        nc.tensor.matmul(
                            psS_t[ac][:],
                            yg_sb[kc][:, q * C + ac * P: q * C + ac * P + P],
                            yf_sb[kc][:, q * C:(q + 1) * C],
                            start=(q == 0 and kc == 0),
                            stop=(q == Q - 1 and kc == 1),
                        )

        # ================================================================
        # YhT = X^T @ Wh^T  in (N, C) layout: 32 row-chunk tiles
        # ================================================================
        yht_sb = [cpool.tile([P, C], f32, name=f"yht{i}") for i in range(2 * Q)]
        for i in range(2 * Q):
            ps = psW.tile([P, C], f32, name="w")
            for kc in range(2):
                nc.tensor.matmul(
                    ps[:],
                    x_sb[kc][:, i * P:(i + 1) * P],
                    wht_sb[kc][:],
                    start=(kc == 0), stop=(kc == 1),
                )
            evac(yht_sb[i][:], ps[:])

        # ================================================================
        # softmax rows of S -> beta (normalized), in SBUF
        # ================================================================
        beta_sb = [cpool.tile([P, C], f32, name=f"beta{ac}") for ac in range(2)]
        for ac in range(2):
            negmax = cpool.tile([P, 1], f32, name=f"negmax{ac}")
            sumexp = cpool.tile([P, 1], f32, name=f"sumexp{ac}")
            rcp = cpool.tile([P, 1], f32, name=f"rcp{ac}")
            expo = cpool.tile([P, C], f32, name=f"expo{ac}")
            nc.vector.tensor_reduce(
                out=negmax[:], in_=psS_t[ac][:],
                axis=mybir.AxisListType.X, op=mybir.AluOpType.max, negate=True)
            nc.scalar.activation(
                expo[:], psS_t[ac][:], AF.Exp,
                bias=negmax[:, 0:1], scale=1.0, accum_out=sumexp[:, 0:1])
            nc.vector.reciprocal(rcp[:], sumexp[:])
            if use_bias:
                nc.scalar.activation(beta_sb[ac][:], expo[:], AF.Copy,
                                     bias=0.0, scale=rcp[:, 0:1])
            else:
                nc.vector.tensor_scalar_mul(beta_sb[ac][:], expo[:], rcp[:, 0:1])

        # ================================================================
        # P^T = beta^T @ Wo^T   (2 tiles (128, C), j' on partitions)
        # ================================================================
        pt_sb = [cpool.tile([P, C], f32, name=f"pt{j}") for j in range(2)]
        for jpc in range(2):
            ps = psW.tile([P, C], f32, name="w")
            for jc in range(2):
                nc.tensor.matmul(
                    ps[:],
                    beta_sb[jc][:, jpc * P:(jpc + 1) * P],
                    wot_sb[jc][:],
                    start=(jc == 0), stop=(jc == 1),
                )
            evac(pt_sb[jpc][:], ps[:])

        # ================================================================
        # Out[o, 16c+q] = (P @ Yh_q^T)[o, c] (+ wosum[o]*bh[c] + bo[o])
        # ================================================================
        for oc in range(2):
            out_sb = cpool.tile([P, C, Q], f32, name=f"out{oc}")
            for q in range(Q):
                ps = psW.tile([P, C], f32, name="w")
                for jc in range(2):
                    nc.tensor.matmul(
                        ps[:],
                        pt_sb[jc][:, oc * P:(oc + 1) * P],
                        yht_sb[2 * q + jc][:],
                        start=(jc == 0),
                        stop=(jc == 1 and not use_bias),
                    )
                if use_bias:
                    # += wosum[o] * bh[c] (rank-1, K=1 matmul), then bo on evac
                    nc.tensor.matmul(
                        ps[:],
                        wosum_row[:, oc * P:(oc + 1) * P],
                        bh_row[:],
                        start=False, stop=True,
                    )
                    nc.scalar.activation(out_sb[:, :, q], ps[:], AF.Identity,
                                         bias=bo_sb[oc], scale=1.0)
                else:
                    evac(out_sb[:, :, q], ps[:])
            nc.sync.dma_start(
                out=out_ext[oc * P:(oc + 1) * P, :],
                in_=out_sb.rearrange("p c q -> p (c q)"),
            )

    return nc


def _split_multiwait_insts(nc, max_waits: int = 1):
    """walrus rejects instructions carrying more than one sync wait; hoist
    extra waits onto same-engine no-ops placed immediately before."""
    from concourse import mybir

    nop_id = 0
    for fn in nc.m.functions:
        for blk in fn.blocks:
            insts = list(blk.instructions)
            new_list = []
            changed = False
            for inst in insts:
                si = inst.sync_info
                if si is not None and len(si.on_wait) > max_waits:
                    waits = list(si.on_wait)
                    for w in waits[:-max_waits]:
                        nop = mybir.InstNoOp(name=f"I-waitnop{nop_id}", ins=[],
                                             outs=[])
                        nop_id += 1
                        nop.engine = inst.engine
                        nop.sync_info = mybir.SyncInfo(on_wait=[w], on_update=[])
                        new_list.append(nop)
                    inst.sync_info = mybir.SyncInfo(
                        on_wait=waits[-max_waits:],
                        on_update=list(si.on_update),
                    )
                    changed = True
                new_list.append(inst)
            if changed:
                blk.instructions = new_list
    return nc


def _get_graph(use_bias: bool):
    key = bool(use_bias)
    if key not in _GRAPH_CACHE:
        _GRAPH_CACHE[key] = _split_multiwait_insts(_build_graph(key))
    return _GRAPH_CACHE[key]


def _make_in_maps(inputs, use_bias):
    x = np.ascontiguousarray(np.asarray(inputs["x"], dtype=np.float32))
    Wf = np.asarray(inputs["Wf"], dtype=np.float32)
    Wg = np.asarray(inputs["Wg"], dtype=np.float32)
    Wh = np.asarray(inputs["Wh"], dtype=np.float32)
    Wo = np.asarray(inputs["Wo"], dtype=np.float32)

    wht = np.ascontiguousarray(Wh.T)
    wot = np.ascontiguousarray(Wo.T)
    if use_bias:
        bf = np.asarray(inputs["bf"], np.float32)
        bg = np.asarray(inputs["bg"], np.float32)
        bh = np.asarray(inputs["bh"], np.float32)
        bo = np.asarray(inputs["bo"], np.float32)
        wpk = np.concatenate([Wf.T, Wg.T, wht, wot], axis=0)
        bpk = np.stack([bf[:P], bf[P:], bg[:P], bg[P:], bo[:P], bo[P:]], axis=1)
        bhw = np.concatenate([bh, Wo.sum(axis=1)]).reshape(1, 2 * C)
        common = {
            "wpk": np.ascontiguousarray(wpk),
            "bpk": np.ascontiguousarray(bpk),
            "bhw": np.ascontiguousarray(bhw),
        }
    else:
        wpk = np.concatenate([Wf.T @ Wg, wht, wot], axis=0)
        common = {"wpk": np.ascontiguousarray(wpk)}

    return [
        {"x": np.ascontiguousarray(x[i].reshape(C, N)), **common}
        for i in range(NCORES)
    ]


def kernel(x, Wf, bf, Wg, bg, Wh, bh, Wo, bo):
    from concourse.bass_utils import run_bass_kernel_spmd

    inputs = {"x": x, "Wf": Wf, "bf": bf, "Wg": Wg, "bg": bg,
              "Wh": Wh, "bh": bh, "Wo": Wo, "bo": bo}
    use_bias = bool(
        np.any(np.asarray(bf)) or np.any(np.asarray(bg))
        or np.any(np.asarray(bh)) or np.any(np.asarray(bo))
    )
    nc = _get_graph(use_bias)
    in_maps = _make_in_maps(inputs, use_bias)
    res = run_bass_kernel_spmd(nc, in_maps, list(range(NCORES)))
    out = np.stack([res.results[i]["out"] for i in range(NCORES)])
    return out.reshape(B, C, W_DIM, H_DIM)


# revision 16
# speedup vs baseline: 1.5436x; 1.5436x over previous
"""Trainium2 Bass kernel for nn_Attention_88493506167116.

Channel-attention module (per batch item b):
    F = (Wf @ X).reshape raw (N, C);  G likewise;  Hm likewise (X = x[b] as (C, N))
    S = G^T @ F  (C x C), beta = softmax(S, axis=-1)
    O = beta @ Hm^T  (C, N) -> reshape (C, W, H);  out = Wo @ O + bo

Key structure (C=256, N=4096 = 16*C): the raw reshape (C, N) -> (N, C) is a
block regrouping: F_r[16c+q, r] = Yf[c, q*C + r].  Hence with X_q = X[:, qC:(q+1)C]:
    S     = sum_q Yg_q^T Yf_q = sum_q X_q^T A X_q,   A = Wg^T Wf   (host-folded)
    out   = Wo @ O: with P = Wo @ beta, Out[o, 16c+q] = (P @ Yh_q^T)[o, c]
and Yh_q^T = YhT[qC:(q+1)C, :] where YhT = X^T Wh^T is computed directly in
transposed layout (no on-device transposes anywhere).

Sharding: pure data-parallel, batch B=8 across the 8 NeuronCores (one image
per core), zero collectives.  Host folds A = Wf.T @ Wg (lhsT layout) and
transposes weights; all heavy compute is on-device fp32 matmuls.

Hardware constraints honored here: fp32 matmuls self-load weights (S3_LW) and
DMA instructions each carry at most ONE sync wait, so (a) all weights are
packed into one DRAM param (<=8 DMAs total -> no DMA sem-lane reuse), (b) tiny
warmup matmuls touch every DMA'd matmul input first, and (c) all PSUM
evacuations / matmul-input producers run on a single engine per path so later
waits coalesce onto one semaphore.
"""

import numpy as np

B, C, W_DIM, H_DIM = 8, 256, 64, 64
N = W_DIM * H_DIM          # 4096
Q = N // C                 # 16
P = 128                    # partitions
NCORES = 8

_GRAPH_CACHE = {}


def _build_graph(use_bias: bool):
    from contextlib import ExitStack

    import concourse.bass as bass
    import concourse.tile as tile
    from concourse import mybir

    f32 = mybir.dt.float32
    f32r = mybir.dt.float32r
    AF = mybir.ActivationFunctionType

    nc = bass.Bass()

    # fast (zero-bias) path computes in float32r: full fp32 exponent/storage,
    # PE rounds products to ~13 mantissa bits but runs 4x faster than fp32.
    # Measured end-to-end error vs fp32 reference ~1e-3. Bias path stays fp32.
    dcomp = f32 if use_bias else f32r

    NW = 8 if use_bias else 6
    x_ext = nc.declare_dram_parameter("x", [C, N], dcomp, isOutput=False)
    wpk_ext = nc.declare_dram_parameter("wpk", [NW * P, C], dcomp, isOutput=False)
    if use_bias:
        bpk_ext = nc.declare_dram_parameter("bpk", [P, 6], f32, isOutput=False)
        bhw_ext = nc.declare_dram_parameter("bhw", [1, 2 * C], f32, isOutput=False)
    out_ext = nc.declare_dram_parameter("out", [C, N], f32, isOutput=True)

    with ExitStack() as ctx:
        tc = ctx.enter_context(tile.TileContext(nc))
        cpool = ctx.enter_context(tc.tile_pool(name="cpool", bufs=1))
        psS = ctx.enter_context(tc.tile_pool(name="psS", bufs=1, space="PSUM"))
        psW = ctx.enter_context(tc.tile_pool(name="psW", bufs=3, space="PSUM"))
        psV = ctx.enter_context(tc.tile_pool(name="psV", bufs=2, space="PSUM"))
        psX = ctx.enter_context(tc.tile_pool(name="psX", bufs=1, space="PSUM"))

        # Evacuation engine split keeps every fp32/f32r matmul at <=1 sync
        # wait: psW slots are only ever read by ACT, psV slots only by DVE,
        # and each stage's matmul inputs come from a single engine.
        if use_bias:
            def evacA(dst, src):
                nc.scalar.copy(dst, src)
            evacV = evacA
        else:
            def evacA(dst, src):
                nc.scalar.copy(dst, src)

            def evacV(dst, src):
                nc.vector.tensor_copy(dst, src)

        # ---- resident SBUF tensors -------------------------------------
        # x loaded in 2 column halves per row-chunk (4 DMAs) so the first
        # compute stages can start before the full 4 MB lands.
        x_sb = [cpool.tile([P, N], dcomp, name=f"x{kc}") for kc in range(2)]
        wpk_sb = cpool.tile([P, NW, C], dcomp, name="wpk")
        NH = N // 2
        for kc in range(2):
            for h in range(2):
                nc.sync.dma_start(
                    out=x_sb[kc][:, h * NH:(h + 1) * NH],
                    in_=x_ext[kc * P:(kc + 1) * P, h * NH:(h + 1) * NH])
        nc.sync.dma_start(out=wpk_sb[:], in_=wpk_ext.rearrange("(g p) c -> p g c", p=P))

        if use_bias:
            wft_sb = [wpk_sb[:, 0 + kc, :] for kc in range(2)]
            wgt_sb = [wpk_sb[:, 2 + kc, :] for kc in range(2)]
            wht_sb = [wpk_sb[:, 4 + kc, :] for kc in range(2)]
            wot_sb = [wpk_sb[:, 6 + kc, :] for kc in range(2)]
            bpk_sb = cpool.tile([P, 6], f32, name="bpk")
            bhw_sb = cpool.tile([1, 2 * C], f32, name="bhw")
            nc.sync.dma_start(out=bpk_sb[:], in_=bpk_ext[:])
            nc.sync.dma_start(out=bhw_sb[:], in_=bhw_ext[:])
            bf_sb = [bpk_sb[:, 0 + kc:1 + kc] for kc in range(2)]
            bg_sb = [bpk_sb[:, 2 + kc:3 + kc] for kc in range(2)]
            bo_sb = [bpk_sb[:, 4 + kc:5 + kc] for kc in range(2)]
            bh_row = bhw_sb[0:1, 0:C]
            wosum_row = bhw_sb[0:1, C:2 * C]
        else:
            at_sb = [wpk_sb[:, 0 + kc, :] for kc in range(2)]
            wht_sb = [wpk_sb[:, 2 + kc, :] for kc in range(2)]
            wot_sb = [wpk_sb[:, 4 + kc, :] for kc in range(2)]

        # ---- PE sync warmup: one tiny matmul per DMA sem lane ----------
        # (fp32/f32r matmuls take <=1 sync wait; these cover the DMA lanes
        # so real matmuls never need two DMA waits at once)
        scratch_ps = psX.tile([P, 1], f32, name="scratch")

        def warmup(t):
            # plain-fp32 bitcast: f32r matmuls have ISA restrictions on tiny
            # moving dims, and these 1-column matmuls exist only for sync
            nc.tensor.matmul(scratch_ps[:], t.bitcast(f32), t[:, 0:1].bitcast(f32),
                             start=True, stop=True)

        warmup(wpk_sb[:, 0, 0:P])
        warmup(x_sb[0][:, 0:P])
        warmup(x_sb[1][:, 0:P])
        if use_bias:
            warmup(x_sb[0][:, NH:NH + P])
            warmup(x_sb[1][:, NH:NH + P])
            nc.tensor.matmul(scratch_ps[0:1, :], bhw_sb[0:1, 0:1],
                             bhw_sb[0:1, 0:1], start=True, stop=True)
            # pre-cover the bpk DMA lane on the ACT engine (bias reads)
            act_scr = cpool.tile([P, 1], f32, name="act_scr")
            nc.scalar.copy(act_scr[:], bpk_sb[:, 0:1])

        # S accumulator PSUM tiles, pinned across the whole contraction
        psS_t = [psS.tile([P, C], f32, name=f"S{ac}") for ac in range(2)]

        # ================================================================
        # Pre-softmax path: fill psS_t[ac] with S = G_r^T F_r
        # ================================================================
        if not use_bias:
            # T_q = A @ X_q (two q at a time, 512-wide), then S += X_q^T T_q
            # (A_T = Wf^T Wg passed in)
            t2_sb = [[cpool.tile([P, 2 * C], dcomp, name=f"t{qp}_{uc}")
                      for uc in range(2)] for qp in range(Q // 2)]
            for qp in range(Q // 2):
                if qp == Q // 4:  # second x column half arrives
                    warmup(x_sb[0][:, NH:NH + P])
                    warmup(x_sb[1][:, NH:NH + P])
                for uc in range(2):
                    ps = psW.tile([P, 2 * C], f32, name="w")
                    for kc in range(2):
                        nc.tensor.matmul(
                            ps[:],
                            at_sb[kc][:, uc * P:(uc + 1) * P],
                            x_sb[kc][:, qp * 2 * C:(qp + 1) * 2 * C],
                            start=(kc == 0), stop=(kc == 1),
                        )
                    evacA(t2_sb[qp][uc][:], ps[:])
            for ac in range(2):
                for q in range(Q):
                    for uc in range(2):
                        nc.tensor.matmul(
                            psS_t[ac][:],
                            x_sb[uc][:, q * C + ac * P: q * C + ac * P + P],
                            t2_sb[q // 2][uc][:, (q % 2) * C:(q % 2 + 1) * C],
                            start=(q == 0 and uc == 0),
                            stop=(q == Q - 1 and uc == 1),
                        )
        else:
            # materialize Yf = Wf X + bf and Yg = Wg X + bg, then
            # S = sum_q Yg_q^T Yf_q
            yf_sb = [cpool.tile([P, N], f32, name=f"yf{mc}") for mc in range(2)]
            yg_sb = [cpool.tile([P, N], f32, name=f"yg{mc}") for mc in range(2)]
            for mc in range(2):
                for nb in range(8):
                    nsl = slice(nb * 512, (nb + 1) * 512)
                    ps = psW.tile([P, 512], f32, name="w")
                    for kc in range(2):
                        nc.tensor.matmul(
                            ps[:], wft_sb[kc][:, mc * P:(mc + 1) * P],
                            x_sb[kc][:, nsl], start=(kc == 0), stop=(kc == 1))
                    nc.scalar.activation(yf_sb[mc][:, nsl], ps[:], AF.Identity,
                                         bias=bf_sb[mc], scale=1.0)
                    ps = psW.tile([P, 512], f32, name="w")
                    for kc in range(2):
                        nc.tensor.matmul(
                            ps[:], wgt_sb[kc][:, mc * P:(mc + 1) * P],
                            x_sb[kc][:, nsl], start=(kc == 0), stop=(kc == 1))
                    nc.scalar.activation(yg_sb[mc][:, nsl], ps[:], AF.Identity,
                                         bias=bg_sb[mc], scale=1.0)
            for ac in range(2):
                for q in range(Q):
                    for kc in range(2):
                        nc.tensor.matmul(
                            psS_t[ac][:],
                            yg_sb[kc][:, q * C + ac * P: q * C + ac * P + P],
                            yf_sb[kc][:, q * C:(q + 1) * C],
                            start=(q == 0 and kc == 0),
                            stop=(q == Q - 1 and kc == 1),
                        )

        # ================================================================
        # YhT = X^T @ Wh^T  in (N, C) layout: 32 row-chunk tiles
        # ================================================================
        yht_sb = [cpool.tile([P, C], dcomp, name=f"yht{i}") for i in range(2 * Q)]
        for i in range(2 * Q):
            ps = psV.tile([P, C], f32, name="v")
            for kc in range(2):
                nc.tensor.matmul(
                    ps[:],
                    x_sb[kc][:, i * P:(i + 1) * P],
                    wht_sb[kc][:],
                    start=(kc == 0), stop=(kc == 1),
                )
            evacV(yht_sb[i][:], ps[:])

        # ================================================================
        # softmax rows of S -> beta (normalized), in SBUF
        # ================================================================
        beta_sb = [cpool.tile([P, C], dcomp, name=f"beta{ac}") for ac in range(2)]
        for ac in range(2):
            negmax = cpool.tile([P, 1], f32, name=f"negmax{ac}")
            sumexp = cpool.tile([P, 1], f32, name=f"sumexp{ac}")
            rcp = cpool.tile([P, 1], f32, name=f"rcp{ac}")
            expo = cpool.tile([P, C], f32, name=f"expo{ac}")
            nc.vector.tensor_reduce(
                out=negmax[:], in_=psS_t[ac][:],
                axis=mybir.AxisListType.X, op=mybir.AluOpType.max, negate=True)
            nc.scalar.activation(
                expo[:], psS_t[ac][:], AF.Exp,
                bias=negmax[:, 0:1], scale=1.0, accum_out=sumexp[:, 0:1])
            nc.vector.reciprocal(rcp[:], sumexp[:])
            if use_bias:
                nc.scalar.activation(beta_sb[ac][:], expo[:], AF.Copy,
                                     bias=0.0, scale=rcp[:, 0:1])
            else:
                nc.vector.tensor_scalar_mul(beta_sb[ac][:], expo[:], rcp[:, 0:1])

        # ================================================================
        # P^T = beta^T @ Wo^T   (2 tiles (128, C), j' on partitions)
        # ================================================================
        pt_sb = [cpool.tile([P, C], dcomp, name=f"pt{j}") for j in range(2)]
        for jpc in range(2):
            ps = psV.tile([P, C], f32, name="v")
            for jc in range(2):
                nc.tensor.matmul(
                    ps[:],
                    beta_sb[jc][:, jpc * P:(jpc + 1) * P],
                    wot_sb[jc][:],
                    start=(jc == 0), stop=(jc == 1),
                )
            evacV(pt_sb[jpc][:], ps[:])

        # ================================================================
        # Out[o, 16c+q] = (P @ Yh_q^T)[o, c] (+ wosum[o]*bh[c] + bo[o])
        # ================================================================
        for oc in range(2):
            out_sb = cpool.tile([P, C, Q], f32, name=f"out{oc}")
            for q in range(Q):
                ps = psV.tile([P, C], f32, name="v")
                for jc in range(2):
                    nc.tensor.matmul(
                        ps[:],
                        pt_sb[jc][:, oc * P:(oc + 1) * P],
                        yht_sb[2 * q + jc][:],
                        start=(jc == 0),
                        stop=(jc == 1 and not use_bias),
                    )
                if use_bias:
                    # += wosum[o] * bh[c] (rank-1, K=1 matmul), then bo on evac
                    nc.tensor.matmul(
                        ps[:],
                        wosum_row[:, oc * P:(oc + 1) * P],
                        bh_row[:],
                        start=False, stop=True,
                    )
                    nc.scalar.activation(out_sb[:, :, q], ps[:], AF.Identity,
                                         bias=bo_sb[oc], scale=1.0)
                else:
                    evacV(out_sb[:, :, q], ps[:])
            nc.sync.dma_start(
                out=out_ext[oc * P:(oc + 1) * P, :],
                in_=out_sb.rearrange("p c q -> p (c q)"),
            )

    return nc


def _split_multiwait_insts(nc, max_waits: int = 1):
    """walrus rejects instructions carrying more than one sync wait; hoist
    extra waits onto same-engine no-ops placed immediately before."""
    from concourse import mybir

    nop_id = 0
    for fn in nc.m.functions:
        for blk in fn.blocks:
            insts = list(blk.instructions)
            new_list = []
            changed = False
            for inst in insts:
                si = inst.sync_info
                if si is not None and len(si.on_wait) > max_waits:
                    waits = list(si.on_wait)
                    for w in waits[:-max_waits]:
                        nop = mybir.InstNoOp(name=f"I-waitnop{nop_id}", ins=[],
                                             outs=[])
                        nop_id += 1
                        nop.engine = inst.engine
                        nop.sync_info = mybir.SyncInfo(on_wait=[w], on_update=[])
                        new_list.append(nop)
                    inst.sync_info = mybir.SyncInfo(
                        on_wait=waits[-max_waits:],
                        on_update=list(si.on_update),
                    )
                    changed = True
                new_list.append(inst)
            if changed:
                blk.instructions = new_list
    return nc


def _get_graph(use_bias: bool):
    key = bool(use_bias)
    if key not in _GRAPH_CACHE:
        _GRAPH_CACHE[key] = _split_multiwait_insts(_build_graph(key))
    return _GRAPH_CACHE[key]


def _make_in_maps(inputs, use_bias):
    x = np.ascontiguousarray(np.asarray(inputs["x"], dtype=np.float32))
    Wf = np.asarray(inputs["Wf"], dtype=np.float32)
    Wg = np.asarray(inputs["Wg"], dtype=np.float32)
    Wh = np.asarray(inputs["Wh"], dtype=np.float32)
    Wo = np.asarray(inputs["Wo"], dtype=np.float32)

    wht = np.ascontiguousarray(Wh.T)
    wot = np.ascontiguousarray(Wo.T)
    if use_bias:
        bf = np.asarray(inputs["bf"], np.float32)
        bg = np.asarray(inputs["bg"], np.float32)
        bh = np.asarray(inputs["bh"], np.float32)
        bo = np.asarray(inputs["bo"], np.float32)
        wpk = np.concatenate([Wf.T, Wg.T, wht, wot], axis=0)
        bpk = np.stack([bf[:P], bf[P:], bg[:P], bg[P:], bo[:P], bo[P:]], axis=1)
        bhw = np.concatenate([bh, Wo.sum(axis=1)]).reshape(1, 2 * C)
        common = {
            "wpk": np.ascontiguousarray(wpk),
            "bpk": np.ascontiguousarray(bpk),
            "bhw": np.ascontiguousarray(bhw),
        }
    else:
        wpk = np.concatenate([Wf.T @ Wg, wht, wot], axis=0)
        common = {"wpk": np.ascontiguousarray(wpk)}

    return [
        {"x": np.ascontiguousarray(x[i].reshape(C, N)), **common}
        for i in range(NCORES)
    ]


def kernel(x, Wf, bf, Wg, bg, Wh, bh, Wo, bo):
    from concourse.bass_utils import run_bass_kernel_spmd

    inputs = {"x": x, "Wf": Wf, "bf": bf, "Wg": Wg, "bg": bg,
              "Wh": Wh, "bh": bh, "Wo": Wo, "bo": bo}
    use_bias = bool(
        np.any(np.asarray(bf)) or np.any(np.asarray(bg))
        or np.any(np.asarray(bh)) or np.any(np.asarray(bo))
    )
    nc = _get_graph(use_bias)
    in_maps = _make_in_maps(inputs, use_bias)
    res = run_bass_kernel_spmd(nc, in_maps, list(range(NCORES)))
    out = np.stack([res.results[i]["out"] for i in range(NCORES)])
    return out.reshape(B, C, W_DIM, H_DIM)


# revision 17
# speedup vs baseline: 2.3164x; 1.5007x over previous
"""Trainium2 Bass kernel for nn_Attention_88493506167116.

Channel-attention module (per batch item b):
    F = (Wf @ X).reshape raw (N, C);  G likewise;  Hm likewise (X = x[b] as (C, N))
    S = G^T @ F  (C x C), beta = softmax(S, axis=-1)
    O = beta @ Hm^T  (C, N) -> reshape (C, W, H);  out = Wo @ O + bo

Key structure (C=256, N=4096 = 16*C): the raw reshape (C, N) -> (N, C) is a
block regrouping: F_r[16c+q, r] = Yf[c, q*C + r].  Hence with X_q = X[:, qC:(q+1)C]:
    S     = sum_q Yg_q^T Yf_q = sum_q X_q^T A X_q,   A = Wg^T Wf   (host-folded)
    out   = Wo @ O: with P = Wo @ beta, Out[o, 16c+q] = (P @ Yh_q^T)[o, c]
and Yh_q^T = YhT[qC:(q+1)C, :] where YhT = X^T Wh^T is computed directly in
transposed layout (no on-device transposes anywhere).

Sharding: pure data-parallel, batch B=8 across the 8 NeuronCores (one image
per core), zero collectives.  Host folds A = Wf.T @ Wg (lhsT layout) and
transposes weights.

The fast (zero-bias) path computes in float32r: fp32 storage/exponent, PE
rounds products to ~13 mantissa bits, 4x faster than fp32 matmul.  Measured
end-to-end error ~1.3e-3.  The general-bias path stays full fp32.

Hardware constraints honored: fp32/f32r matmuls self-load weights (S3_LW)
and, like DMA instructions, carry at most ONE sync wait.  So: weights packed
into one contiguous DMA; tiny fp32 warmup matmuls cover each DMA sem lane
before first use; PSUM pools are split so each pool's tiles are only ever
read by one engine (psW -> ACT, psV -> DVE), making every matmul's WAR +
input waits coalesce onto a single semaphore.  A post-pass splits any
residual multi-wait instruction (the tail drain) into single-wait no-ops.
"""

import numpy as np

B, C, W_DIM, H_DIM = 8, 256, 64, 64
N = W_DIM * H_DIM          # 4096
Q = N // C                 # 16
P = 128                    # partitions
NCORES = 8

_GRAPH_CACHE = {}


def _build_graph(use_bias: bool):
    from contextlib import ExitStack

    import concourse.bass as bass
    import concourse.tile as tile
    from concourse import mybir

    f32 = mybir.dt.float32
    f32r = mybir.dt.float32r
    AF = mybir.ActivationFunctionType

    nc = bass.Bass()

    dcomp = f32 if use_bias else f32r

    NW = 8 if use_bias else 6
    x_ext = nc.declare_dram_parameter("x", [C, N], dcomp, isOutput=False)
    # wpk is pre-swizzled on host to (P, NW*C): partition-major, contiguous
    # 6KB lines per partition -> fast DMA (vs 768 separate 1KB descriptors)
    wpk_ext = nc.declare_dram_parameter("wpk", [P, NW * C], dcomp, isOutput=False)
    if use_bias:
        bpk_ext = nc.declare_dram_parameter("bpk", [P, 6], f32, isOutput=False)
        bhw_ext = nc.declare_dram_parameter("bhw", [1, 2 * C], f32, isOutput=False)
    out_ext = nc.declare_dram_parameter("out", [C, N], f32, isOutput=True)

    with ExitStack() as ctx:
        tc = ctx.enter_context(tile.TileContext(nc))
        cpool = ctx.enter_context(tc.tile_pool(name="cpool", bufs=1))
        psS = ctx.enter_context(tc.tile_pool(name="psS", bufs=1, space="PSUM"))
        psW = ctx.enter_context(tc.tile_pool(name="psW", bufs=3, space="PSUM"))
        psV = ctx.enter_context(tc.tile_pool(name="psV", bufs=3, space="PSUM"))

        # pool-consistent evacuation engines: psW tiles are read only by the
        # scalar engine (ACT), psV tiles only by the vector engine (DVE)
        def evacA(dst, src):
            nc.scalar.copy(dst, src)

        def evacV(dst, src):
            nc.vector.tensor_copy(dst, src)

        if use_bias:
            evacV = evacA  # single engine keeps the wait discipline trivial

        def pick(i):
            """alternate (pool, evac) by index for load balance"""
            if use_bias:
                return psW, evacA
            return (psV, evacV) if i % 2 == 0 else (psW, evacA)

        # ---- resident SBUF tensors -------------------------------------
        # x loaded in 4 column quarters per row-chunk so compute can start
        # after the first ~1 MB lands.
        x_sb = [cpool.tile([P, N], dcomp, name=f"x{kc}") for kc in range(2)]
        wpk_sb = cpool.tile([P, NW, C], dcomp, name="wpk")
        nc.sync.dma_start(out=wpk_sb.rearrange("p a b -> p (a b)"), in_=wpk_ext[:])
        NQT = N // 4
        for h in range(4):
            for kc in range(2):
                nc.sync.dma_start(
                    out=x_sb[kc][:, h * NQT:(h + 1) * NQT],
                    in_=x_ext[kc * P:(kc + 1) * P, h * NQT:(h + 1) * NQT])

        if use_bias:
            wft_sb = [wpk_sb[:, 0 + kc, :] for kc in range(2)]
            wgt_sb = [wpk_sb[:, 2 + kc, :] for kc in range(2)]
            wht_sb = [wpk_sb[:, 4 + kc, :] for kc in range(2)]
            wot_sb = [wpk_sb[:, 6 + kc, :] for kc in range(2)]
            bpk_sb = cpool.tile([P, 6], f32, name="bpk")
            bhw_sb = cpool.tile([1, 2 * C], f32, name="bhw")
            nc.sync.dma_start(out=bpk_sb[:], in_=bpk_ext[:])
            nc.sync.dma_start(out=bhw_sb[:], in_=bhw_ext[:])
            bf_sb = [bpk_sb[:, 0 + kc:1 + kc] for kc in range(2)]
            bg_sb = [bpk_sb[:, 2 + kc:3 + kc] for kc in range(2)]
            bo_sb = [bpk_sb[:, 4 + kc:5 + kc] for kc in range(2)]
            bh_row = bhw_sb[0:1, 0:C]
            wosum_row = bhw_sb[0:1, C:2 * C]
        else:
            at_sb = [wpk_sb[:, 0 + kc, :] for kc in range(2)]
            wht_sb = [wpk_sb[:, 2 + kc, :] for kc in range(2)]
            wot_sb = [wpk_sb[:, 4 + kc, :] for kc in range(2)]

        # ---- PE sync warmup (one tiny fp32 matmul per DMA sem lane) ----
        scratch_ps = psV.tile([P, 512], f32, name="v")

        def warmup(t):
            nc.tensor.matmul(scratch_ps[:, 0:1], t.bitcast(f32),
                             t[:, 0:1].bitcast(f32), start=True, stop=True)

        warmup(wpk_sb[:, 0, 0:P])
        warmup(x_sb[0][:, 0:P])
        warmup(x_sb[1][:, 0:P])
        if use_bias:
            for h in range(1, 4):
                warmup(x_sb[0][:, h * NQT:h * NQT + P])
                warmup(x_sb[1][:, h * NQT:h * NQT + P])
            nc.tensor.matmul(scratch_ps[0:1, 0:1], bhw_sb[0:1, 0:1],
                             bhw_sb[0:1, 0:1], start=True, stop=True)
            act_scr = cpool.tile([P, 1], f32, name="act_scr")
            nc.scalar.copy(act_scr[:], bpk_sb[:, 0:1])

        # S accumulator PSUM tiles, pinned across the whole contraction
        psS_t = [psS.tile([P, C], f32, name=f"S{ac}") for ac in range(2)]

        # ================================================================
        # Pre-softmax path: fill psS_t[ac] with S = G_r^T F_r
        # ================================================================
        if not use_bias:
            # T_q = A @ X_q (two q at a time, 512-wide), then S += X_q^T T_q
            t2_sb = [[cpool.tile([P, 2 * C], dcomp, name=f"t{qp}_{uc}")
                      for uc in range(2)] for qp in range(Q // 2)]
            for qp in range(Q // 2):
                if qp > 0 and qp % 2 == 0:  # next x column quarter arrives
                    warmup(x_sb[0][:, qp * 2 * C:qp * 2 * C + P])
                    warmup(x_sb[1][:, qp * 2 * C:qp * 2 * C + P])
                for uc in range(2):
                    pool, ev = pick(qp * 2 + uc)
                    ps = pool.tile([P, 2 * C], f32,
                                   name="v" if pool is psV else "w")
                    for kc in range(2):
                        nc.tensor.matmul(
                            ps[:],
                            at_sb[kc][:, uc * P:(uc + 1) * P],
                            x_sb[kc][:, qp * 2 * C:(qp + 1) * 2 * C],
                            start=(kc == 0), stop=(kc == 1),
                        )
                    ev(t2_sb[qp][uc][:], ps[:])
            for ac in range(2):
                for q in range(Q):
                    for uc in range(2):
                        nc.tensor.matmul(
                            psS_t[ac][:],
                            x_sb[uc][:, q * C + ac * P: q * C + ac * P + P],
                            t2_sb[q // 2][uc][:, (q % 2) * C:(q % 2 + 1) * C],
                            start=(q == 0 and uc == 0),
                            stop=(q == Q - 1 and uc == 1),
                        )
        else:
            # materialize Yf = Wf X + bf and Yg = Wg X + bg, then
            # S = sum_q Yg_q^T Yf_q
            yf_sb = [cpool.tile([P, N], f32, name=f"yf{mc}") for mc in range(2)]
            yg_sb = [cpool.tile([P, N], f32, name=f"yg{mc}") for mc in range(2)]
            for mc in range(2):
                for nb in range(8):
                    nsl = slice(nb * 512, (nb + 1) * 512)
                    ps = psW.tile([P, 512], f32, name="w")
                    for kc in range(2):
                        nc.tensor.matmul(
                            ps[:], wft_sb[kc][:, mc * P:(mc + 1) * P],
                            x_sb[kc][:, nsl], start=(kc == 0), stop=(kc == 1))
                    nc.scalar.activation(yf_sb[mc][:, nsl], ps[:], AF.Identity,
                                         bias=bf_sb[mc], scale=1.0)
                    ps = psW.tile([P, 512], f32, name="w")
                    for kc in range(2):
                        nc.tensor.matmul(
                            ps[:], wgt_sb[kc][:, mc * P:(mc + 1) * P],
                            x_sb[kc][:, nsl], start=(kc == 0), stop=(kc == 1))
                    nc.scalar.activation(yg_sb[mc][:, nsl], ps[:], AF.Identity,
                                         bias=bg_sb[mc], scale=1.0)
            for ac in range(2):
                for q in range(Q):
                    for kc in range(2):
                        nc.tensor.matmul(
                            psS_t[ac][:],
                            yg_sb[kc][:, q * C + ac * P: q * C + ac * P + P],
                            yf_sb[kc][:, q * C:(q + 1) * C],
                            start=(q == 0 and kc == 0),
                            stop=(q == Q - 1 and kc == 1),
                        )

        # ================================================================
        # YhT = X^T @ Wh^T in (N, C) layout: 16 paired tiles (128, 512),
        # pair i holds row-chunks 2i (cols 0:C) and 2i+1 (cols C:2C)
        # ================================================================
        yht_sb = [cpool.tile([P, 2 * C], dcomp, name=f"yht{i}") for i in range(Q)]
        for i in range(Q):
            pool, ev = pick(i)
            ps = pool.tile([P, 2 * C], f32, name="v" if pool is psV else "w")
            for half in range(2):
                nch = 2 * i + half
                for kc in range(2):
                    nc.tensor.matmul(
                        ps[:, half * C:(half + 1) * C],
                        x_sb[kc][:, nch * P:(nch + 1) * P],
                        wht_sb[kc][:],
                        start=(kc == 0), stop=(kc == 1),
                    )
            ev(yht_sb[i][:], ps[:])

        # ================================================================
        # softmax rows of S -> beta (normalized), in SBUF
        # ================================================================
        beta_sb = [cpool.tile([P, C], dcomp, name=f"beta{ac}") for ac in range(2)]
        for ac in range(2):
            negmax = cpool.tile([P, 1], f32, name=f"negmax{ac}")
            sumexp = cpool.tile([P, 1], f32, name=f"sumexp{ac}")
            rcp = cpool.tile([P, 1], f32, name=f"rcp{ac}")
            expo = cpool.tile([P, C], f32, name=f"expo{ac}")
            nc.vector.tensor_reduce(
                out=negmax[:], in_=psS_t[ac][:],
                axis=mybir.AxisListType.X, op=mybir.AluOpType.max, negate=True)
            nc.scalar.activation(
                expo[:], psS_t[ac][:], AF.Exp,
                bias=negmax[:, 0:1], scale=1.0, accum_out=sumexp[:, 0:1])
            nc.vector.reciprocal(rcp[:], sumexp[:])
            if use_bias:
                nc.scalar.activation(beta_sb[ac][:], expo[:], AF.Copy,
                                     bias=0.0, scale=rcp[:, 0:1])
            else:
                nc.vector.tensor_scalar_mul(beta_sb[ac][:], expo[:], rcp[:, 0:1])

        # ================================================================
        # P^T = beta^T @ Wo^T   (2 tiles (128, C), j' on partitions)
        # ================================================================
        pt_sb = [cpool.tile([P, C], dcomp, name=f"pt{j}") for j in range(2)]
        for jpc in range(2):
            pool = psW if use_bias else psV
            ps = pool.tile([P, 2 * C], f32, name="w" if use_bias else "v")
            for jc in range(2):
                nc.tensor.matmul(
                    ps[:, 0:C],
                    beta_sb[jc][:, jpc * P:(jpc + 1) * P],
                    wot_sb[jc][:],
                    start=(jc == 0), stop=(jc == 1),
                )
            (evacA if use_bias else evacV)(pt_sb[jpc][:], ps[:, 0:C])

        # ================================================================
        # Out[o, 16c+q] = (P @ Yh_q^T)[o, c] (+ wosum[o]*bh[c] + bo[o])
        # Two q per PSUM tile; one paired (transposing-AP) evacuation.
        # ================================================================
        for oc in range(2):
            out_sb = cpool.tile([P, C, Q], f32, name=f"out{oc}")
            for u in range(Q // 2):
                pool, ev = pick(u + oc)
                ps = pool.tile([P, 2 * C], f32, name="v" if pool is psV else "w")
                for half in range(2):
                    q = 2 * u + half
                    for jc in range(2):
                        nc.tensor.matmul(
                            ps[:, half * C:(half + 1) * C],
                            pt_sb[jc][:, oc * P:(oc + 1) * P],
                            yht_sb[q][:, jc * C:(jc + 1) * C],
                            start=(jc == 0),
                            stop=(jc == 1 and not use_bias),
                        )
                    if use_bias:
                        nc.tensor.matmul(
                            ps[:, half * C:(half + 1) * C],
                            wosum_row[:, oc * P:(oc + 1) * P],
                            bh_row[:],
                            start=False, stop=True,
                        )
                if use_bias:
                    nc.scalar.activation(
                        out_sb[:, :, 2 * u:2 * u + 2],
                        ps.rearrange("p (h c) -> p c h", h=2),
                        AF.Identity, bias=bo_sb[oc], scale=1.0)
                else:
                    ev(out_sb[:, :, 2 * u:2 * u + 2],
                       ps.rearrange("p (h c) -> p c h", h=2))
            nc.sync.dma_start(
                out=out_ext[oc * P:(oc + 1) * P, :],
                in_=out_sb.rearrange("p c q -> p (c q)"),
            )

    return nc


def _split_multiwait_insts(nc, max_waits: int = 1):
    """walrus rejects instructions carrying more than one sync wait; hoist
    extra waits onto same-engine no-ops placed immediately before."""
    from concourse import mybir

    nop_id = 0
    for fn in nc.m.functions:
        for blk in fn.blocks:
            insts = list(blk.instructions)
            new_list = []
            changed = False
            for inst in insts:
                si = inst.sync_info
                if si is not None and len(si.on_wait) > max_waits:
                    waits = list(si.on_wait)
                    for w in waits[:-max_waits]:
                        nop = mybir.InstNoOp(name=f"I-waitnop{nop_id}", ins=[],
                                             outs=[])
                        nop_id += 1
                        nop.engine = inst.engine
                        nop.sync_info = mybir.SyncInfo(on_wait=[w], on_update=[])
                        new_list.append(nop)
                    inst.sync_info = mybir.SyncInfo(
                        on_wait=waits[-max_waits:],
                        on_update=list(si.on_update),
                    )
                    changed = True
                new_list.append(inst)
            if changed:
                blk.instructions = new_list
    return nc


def _get_graph(use_bias: bool):
    key = bool(use_bias)
    if key not in _GRAPH_CACHE:
        _GRAPH_CACHE[key] = _split_multiwait_insts(_build_graph(key))
    return _GRAPH_CACHE[key]


def _make_in_maps(inputs, use_bias):
    x = np.ascontiguousarray(np.asarray(inputs["x"], dtype=np.float32))
    Wf = np.asarray(inputs["Wf"], dtype=np.float32)
    Wg = np.asarray(inputs["Wg"], dtype=np.float32)
    Wh = np.asarray(inputs["Wh"], dtype=np.float32)
    Wo = np.asarray(inputs["Wo"], dtype=np.float32)

    wht = np.ascontiguousarray(Wh.T)
    wot = np.ascontiguousarray(Wo.T)

    def swizzle(wlist):
        # stack (NW, 128, C) row-chunks then move partitions outermost:
        # wpk[p, g*C:(g+1)*C] = chunk g row p  ->  shape (P, NW*C)
        chunks = []
        for w in wlist:
            chunks.append(w[:P])
            chunks.append(w[P:])
        arr = np.stack(chunks, axis=0)           # (NW, P, C)
        return np.ascontiguousarray(
            arr.transpose(1, 0, 2).reshape(P, -1))

    if use_bias:
        bf = np.asarray(inputs["bf"], np.float32)
        bg = np.asarray(inputs["bg"], np.float32)
        bh = np.asarray(inputs["bh"], np.float32)
        bo = np.asarray(inputs["bo"], np.float32)
        wpk = swizzle([Wf.T, Wg.T, wht, wot])
        bpk = np.stack([bf[:P], bf[P:], bg[:P], bg[P:], bo[:P], bo[P:]], axis=1)
        bhw = np.concatenate([bh, Wo.sum(axis=1)]).reshape(1, 2 * C)
        common = {
            "wpk": wpk,
            "bpk": np.ascontiguousarray(bpk),
            "bhw": np.ascontiguousarray(bhw),
        }
    else:
        wpk = swizzle([Wf.T @ Wg, wht, wot])
        common = {"wpk": wpk}

    return [
        {"x": np.ascontiguousarray(x[i].reshape(C, N)), **common}
        for i in range(NCORES)
    ]


def kernel(x, Wf, bf, Wg, bg, Wh, bh, Wo, bo):
    from concourse.bass_utils import run_bass_kernel_spmd

    inputs = {"x": x, "Wf": Wf, "bf": bf, "Wg": Wg, "bg": bg,
              "Wh": Wh, "bh": bh, "Wo": Wo, "bo": bo}
    use_bias = bool(
        np.any(np.asarray(bf)) or np.any(np.asarray(bg))
        or np.any(np.asarray(bh)) or np.any(np.asarray(bo))
    )
    nc = _get_graph(use_bias)
    in_maps = _make_in_maps(inputs, use_bias)
    res = run_bass_kernel_spmd(nc, in_maps, list(range(NCORES)))
    out = np.stack([res.results[i]["out"] for i in range(NCORES)])
    return out.reshape(B, C, W_DIM, H_DIM)
